# revision 6
# baseline (speedup 1.0000x reference)
"""Trainium2 Bass kernel for FCOSMultiStrideFilter (nms_detection).

Per-level / per-image filter: channel-max + threshold mask + argmax +
masked score/bbox/centerness gather, with the [C, H*W] -> [H*W, C]
layout transpose done on the PE (transpose mode, exact fp32 bit
movement).

Sharding: pure data parallel over the batch dim — core b processes
image b for all 5 FPN levels.

Device layout per core/level (hw positions, CH = 80 cls + 4 bbox + 1 ctr
channels stacked on SBUF partitions):
  - host concatenates cls/bbox/ctr into one [85, hw] tensor per level
  - positions processed in "strided chunks" of 128*K: tile t holds
    positions {pos0 + p*K + t | p in 0..127} so that partition p owns a
    contiguous run of K positions -> all DMA-out writes are
    per-partition contiguous.
  - PE transposes [85, 128] -> PSUM [128, 85] (groups of 5 tiles per
    PSUM bank), ACT evacuates PSUM->SBUF, DVE does the channel-max
    reduce + argmax (max_index), GPSIMD applies the threshold mask and
    the small mid/mv fixups.
  - coord output is input-independent -> computed host-side.
"""

import os
import sys

import numpy as np

_EXTRA_PATHS = [
    "/root/.axon_site",
    "/root/.axon_site/_ro/trn_rl_repo",
    "/root/.axon_site/_ro/pypackages",
    "/opt/trn_rl_repo",
    "/opt/pypackages",
]
for _p in _EXTRA_PATHS:
    if os.path.isdir(_p) and _p not in sys.path:
        sys.path.append(_p)

import concourse.bass as bass  # noqa: E402
import concourse.mybir as mybir  # noqa: E402
from concourse import masks  # noqa: E402
from concourse.bass_utils import run_bass_kernel_spmd  # noqa: E402
from concourse.tile import TileContext  # noqa: E402
from concourse.vector_clock import ScopedClock  # noqa: E402

F32 = mybir.dt.float32
I32 = mybir.dt.int32
U32 = mybir.dt.uint32

B = 8
C = 80
CH = 85  # 80 cls + 4 bbox + 1 ctr rows
LEVELS = [(160, 160), (80, 80), (40, 40), (20, 20), (10, 10)]
THRESHOLD = 0.5
KMAX = 25  # tiles per strided chunk (25*128 = 3200 positions)
GROUP = 5  # transpose tiles per PSUM bank (5*85*4B = 1700B <= 2KB)


# ---------------------------------------------------------------------------
# TileContext final-drain workaround: this walrus build only accepts ONE
# sync-wait command per CTRL instruction, but the Tile exit drain waits on
# every outstanding proc semaphore at once. Split the waits across SP nops.
# ---------------------------------------------------------------------------
_MAX_WAITS_PER_INST = 1


def _drain_and_barrier_split(self, tick_clock, wait_clock):
    nc = self.nc
    nops = [nc.sync.nop(nofuse=True, hint=f"drain_wait_split_{i}") for i in range(32)]
    drain_inst = nc.sync.drain()
    wait_clock.add_sem_waits(
        drain_inst.ins, ScopedClock({None: tick_clock.global_clock})
    )
    si = drain_inst.ins.sync_info
    if si is not None and si.on_wait and len(si.on_wait) > _MAX_WAITS_PER_INST:
        waits = list(si.on_wait)
        si.on_wait = waits[:_MAX_WAITS_PER_INST]
        extra = waits[_MAX_WAITS_PER_INST:]
        k = 0
        while extra:
            chunk, extra = (
                extra[:_MAX_WAITS_PER_INST],
                extra[_MAX_WAITS_PER_INST:],
            )
            nsi = nops[k].ins.sync_info
            if nsi is None:
                nops[k].ins.sync_info = mybir.SyncInfo(on_wait=chunk, on_update=[])
            else:
                nsi.on_wait = (nsi.on_wait or []) + chunk
            k += 1
            if k >= len(nops):
                raise RuntimeError("too many outstanding sem waits to split")

    nc.all_engine_barrier()
    assert self.sems is not None
    popped = nc._tile_sem_poison_stack.pop()
    assert popped is self._sem_poison
    nc.clear_and_free_semaphores(list(self.sems.allocated().values()))
    nc.all_engine_barrier()


def _install_tile_patch():
    from concourse import tile as _tile

    _tile.TileContext._drain_and_barrier = _drain_and_barrier_split


def _split_multi_waits(nc):
    """This walrus build accepts only one sync-wait command per instruction.
    Move extra waits onto same-engine nops inserted immediately before the
    offending instruction."""
    for f in nc.m.functions:
        for bb in f.blocks:
            insts = bb.instructions
            i = 0
            while i < len(insts):
                inst = insts[i]
                si = inst.sync_info
                if si is not None and si.on_wait and len(si.on_wait) > 1:
                    waits = list(si.on_wait)
                    si.on_wait = waits[:1]
                    extra = waits[1:]
                    for j, wt in enumerate(extra):
                        bi = nc.engines[inst.engine].nop(
                            nofuse=True, hint=f"wait_split_{inst.name}_{j}"
                        )
                        nop_inst = bi.ins
                        nop_inst.sync_info = mybir.SyncInfo(
                            on_wait=[wt], on_update=[]
                        )
                        # the nop was appended to nc.cur_bb; relocate it to
                        # just before `inst` in this block
                        cur = nc.cur_bb.bb if hasattr(nc.cur_bb, "bb") else None
                        src_list = cur.instructions if cur is not None else None
                        if src_list is not None and src_list and src_list[-1] is nop_inst:
                            src_list.pop()
                        elif insts and insts[-1] is nop_inst:
                            insts.pop()
                        insts.insert(i, nop_inst)
                        i += 1
                i += 1


def _plan_chunks(hw):
    """Split hw positions into strided chunks (kind 's', pos0, K) covering
    128*K positions each, plus at most one contiguous tail ('t', pos0, w)."""
    plans = []
    pos = 0
    while hw - pos >= 128 * KMAX:
        plans.append(("s", pos, KMAX))
        pos += 128 * KMAX
    rem = hw - pos
    if rem >= 128:
        k = rem // 128
        plans.append(("s", pos, k))
        pos += 128 * k
        rem = hw - pos
    if rem:
        plans.append(("t", pos, rem))
    return plans


def _groups(k):
    return [(g0, min(GROUP, k - g0)) for g0 in range(0, k, GROUP)]


def build_bass():
    _install_tile_patch()
    nc = bass.Bass()

    xs, mvs, mids, scores, bbs, cts = [], [], [], [], [], []
    for l, (h, w) in enumerate(LEVELS):
        hw = h * w
        xs.append(nc.declare_dram_parameter(f"x{l}", [CH, hw], F32, isOutput=False))
    for l, (h, w) in enumerate(LEVELS):
        hw = h * w
        mvs.append(nc.declare_dram_parameter(f"mv{l}", [hw], F32, isOutput=True))
        mids.append(nc.declare_dram_parameter(f"mid{l}", [hw], I32, isOutput=True))
        scores.append(
            nc.declare_dram_parameter(f"score{l}", [hw, C], F32, isOutput=True)
        )
        bbs.append(nc.declare_dram_parameter(f"bb{l}", [hw, 4], F32, isOutput=True))
        cts.append(nc.declare_dram_parameter(f"ct{l}", [hw, 1], F32, isOutput=True))

    with TileContext(nc) as tc:
        with (
            tc.tile_pool(name="const", bufs=1) as constp,
            tc.tile_pool(name="xin", bufs=3) as xinp,
            tc.tile_pool(name="stage", bufs=3) as stagep,
            tc.tile_pool(name="small", bufs=3) as smallp,
            tc.tile_pool(name="psum", bufs=6, space="PSUM") as psump,
        ):
            ident = constp.tile([128, 128], F32)
            masks.make_identity(nc, ident[:])

            for l, (h, w) in enumerate(LEVELS):
                hw = h * w
                for kind, pos0, kk in _plan_chunks(hw):
                    if kind == "s":
                        _emit_strided_chunk(
                            nc, xinp, stagep, smallp, psump, ident,
                            xs[l], mvs[l], mids[l], scores[l], bbs[l], cts[l],
                            pos0, kk,
                        )
                    else:
                        _emit_tail(
                            nc, xinp, stagep, smallp, psump, ident,
                            xs[l], mvs[l], mids[l], scores[l], bbs[l], cts[l],
                            pos0, kk,
                        )
    _split_multi_waits(nc)
    return nc


def _emit_common(nc, stage, mvb, mkb, idx8, idxf, midi, mvm, groups, pp, k):
    """Shared per-chunk compute. `pp` = partition count (128 for strided
    chunks, tail width for tails). Tiles are [pp, k, ...] slices."""
    # per-group: reduce-max over cls channels, threshold mask, argmax,
    # in-place mask multiply of the staged (transposed) data
    for g0, ng in groups:
        nc.vector.tensor_reduce(
            out=mvb[:pp, g0 : g0 + ng, 0],
            in_=stage[:pp, g0 : g0 + ng, 0:C],
            axis=mybir.AxisListType.X,
            op=mybir.AluOpType.max,
        )
        nc.gpsimd.tensor_scalar(
            out=mkb[:pp, g0 : g0 + ng, 0],
            in0=mvb[:pp, g0 : g0 + ng, 0],
            scalar1=THRESHOLD,
            scalar2=None,
            op0=mybir.AluOpType.is_ge,
        )
        for j in range(ng):
            t = g0 + j
            nc.vector.max_index(
                out=idx8[:pp, t, :],
                in_max=mvb[:pp, t, :].to_broadcast([pp, 8]),
                in_values=stage[:pp, t, 0:C],
            )
        nc.gpsimd.tensor_tensor(
            out=stage[:pp, g0 : g0 + ng, :],
            in0=stage[:pp, g0 : g0 + ng, :],
            in1=mkb[:pp, g0 : g0 + ng, :].to_broadcast([pp, ng, CH]),
            op=mybir.AluOpType.mult,
        )
    # batched small fixups over the whole chunk
    nc.gpsimd.tensor_tensor(
        out=mvm[:pp, 0:k, 0],
        in0=mvb[:pp, 0:k, 0],
        in1=mkb[:pp, 0:k, 0],
        op=mybir.AluOpType.mult,
    )
    nc.gpsimd.tensor_copy(out=idxf[:pp, 0:k, 0], in_=idx8[:pp, 0:k, 0])
    nc.gpsimd.tensor_tensor(
        out=idxf[:pp, 0:k, 0],
        in0=idxf[:pp, 0:k, 0],
        in1=mkb[:pp, 0:k, 0],
        op=mybir.AluOpType.mult,
    )
    nc.gpsimd.tensor_tensor(
        out=idxf[:pp, 0:k, 0],
        in0=idxf[:pp, 0:k, 0],
        in1=mkb[:pp, 0:k, 0],
        op=mybir.AluOpType.add,
    )
    # mid = idx*mask + mask - 1  (== idx where mask, -1 where not)
    nc.gpsimd.tensor_scalar(
        out=midi[:pp, 0:k, 0],
        in0=idxf[:pp, 0:k, 0],
        scalar1=-1.0,
        scalar2=None,
        op0=mybir.AluOpType.add,
    )


def _alloc_small(smallp):
    mvb = smallp.tile([128, KMAX, 1], F32, tag="mvb")
    mkb = smallp.tile([128, KMAX, 1], F32, tag="mkb")
    idx8 = smallp.tile([128, KMAX, 8], U32, tag="idx8")
    idxf = smallp.tile([128, KMAX, 1], F32, tag="idxf")
    midi = smallp.tile([128, KMAX, 1], I32, tag="midi")
    mvm = smallp.tile([128, KMAX, 1], F32, tag="mvm")
    return mvb, mkb, idx8, idxf, midi, mvm


def _emit_strided_chunk(
    nc, xinp, stagep, smallp, psump, ident, x, mv, mid, score, bb, ct, pos0, k
):
    cols = 128 * k
    xt = xinp.tile([CH, 128 * KMAX], F32, tag="xt")
    nc.sync.dma_start(out=xt[:, 0:cols], in_=x[:, pos0 : pos0 + cols])
    # view so that [:, t, :] is the [CH, 128] tile with position stride k
    xtv = xt[:, 0:cols].rearrange("c (p k) -> c k p", k=k)

    stage = stagep.tile([128, KMAX, CH], F32, tag="stage")
    mvb, mkb, idx8, idxf, midi, mvm = _alloc_small(smallp)

    groups = _groups(k)
    for g0, ng in groups:
        ps = psump.tile([128, GROUP, CH], F32, tag="ps")
        for j in range(ng):
            t = g0 + j
            nc.tensor.transpose(ps[:, j], xtv[:, t, :], ident[0:CH, 0:CH])
        nc.scalar.copy(out=stage[:, g0 : g0 + ng, :], in_=ps[:, 0:ng, :])

    _emit_common(nc, stage, mvb, mkb, idx8, idxf, midi, mvm, groups, 128, k)

    score_ap = score[pos0 : pos0 + cols, :].rearrange("(p k) c -> p k c", p=128)
    nc.sync.dma_start(out=score_ap, in_=stage[:, 0:k, 0:C])
    bb_ap = bb[pos0 : pos0 + cols, :].rearrange("(p k) c -> p k c", p=128)
    nc.sync.dma_start(out=bb_ap, in_=stage[:, 0:k, C : C + 4])
    ct_ap = ct[pos0 : pos0 + cols, :].rearrange("(p k) c -> p k c", p=128)
    nc.sync.dma_start(out=ct_ap, in_=stage[:, 0:k, C + 4 : CH])
    mv_ap = mv[pos0 : pos0 + cols].rearrange("(p k) -> p k", p=128)
    nc.scalar.dma_start(out=mv_ap, in_=mvm[:, 0:k, 0])
    mid_ap = mid[pos0 : pos0 + cols].rearrange("(p k) -> p k", p=128)
    nc.scalar.dma_start(out=mid_ap, in_=midi[:, 0:k, 0])


def _emit_tail(
    nc, xinp, stagep, smallp, psump, ident, x, mv, mid, score, bb, ct, pos0, w
):
    xt = xinp.tile([CH, 128 * KMAX], F32, tag="xt")
    nc.sync.dma_start(out=xt[:, 0:w], in_=x[:, pos0 : pos0 + w])

    stage = stagep.tile([128, KMAX, CH], F32, tag="stage")
    mvb, mkb, idx8, idxf, midi, mvm = _alloc_small(smallp)

    ps = psump.tile([128, GROUP, CH], F32, tag="ps")
    nc.tensor.transpose(ps[0:w, 0], xt[:, 0:w], ident[0:CH, 0:CH])
    nc.scalar.copy(out=stage[0:w, 0, :], in_=ps[0:w, 0, :])

    _emit_common(nc, stage, mvb, mkb, idx8, idxf, midi, mvm, [(0, 1)], w, 1)

    nc.sync.dma_start(out=score[pos0 : pos0 + w, :], in_=stage[0:w, 0, 0:C])
    nc.sync.dma_start(out=bb[pos0 : pos0 + w, :], in_=stage[0:w, 0, C : C + 4])
    nc.sync.dma_start(out=ct[pos0 : pos0 + w, :], in_=stage[0:w, 0, C + 4 : CH])
    mv_ap = mv[pos0 : pos0 + w].rearrange("(p k) -> p k", k=1)
    nc.scalar.dma_start(out=mv_ap, in_=mvm[0:w, 0:1, 0])
    mid_ap = mid[pos0 : pos0 + w].rearrange("(p k) -> p k", k=1)
    nc.scalar.dma_start(out=mid_ap, in_=midi[0:w, 0:1, 0])


_NC_CACHE = None


def _get_nc():
    global _NC_CACHE
    if _NC_CACHE is None:
        _NC_CACHE = build_bass()
    return _NC_CACHE


def kernel(**inputs):
    nc = _get_nc()

    in_maps = []
    for b in range(B):
        m = {}
        for l, (h, w) in enumerate(LEVELS):
            hw = h * w
            m[f"x{l}"] = np.ascontiguousarray(
                np.concatenate(
                    [
                        np.asarray(inputs[f"cls{l}"][b], np.float32).reshape(C, hw),
                        np.asarray(inputs[f"bbox{l}"][b], np.float32).reshape(4, hw),
                        np.asarray(inputs[f"ctr{l}"][b], np.float32).reshape(1, hw),
                    ],
                    axis=0,
                )
            )
        in_maps.append(m)

    res = run_bass_kernel_spmd(nc, in_maps, list(range(B))).results

    out = []
    for l, (h, w) in enumerate(LEVELS):
        hw = h * w
        mv = np.stack([res[b][f"mv{l}"].reshape(hw) for b in range(B)])
        mid = np.stack(
            [res[b][f"mid{l}"].reshape(hw).astype(np.int32) for b in range(B)]
        )
        score = np.stack([res[b][f"score{l}"].reshape(hw, C) for b in range(B)])
        bb = np.stack([res[b][f"bb{l}"].reshape(hw, 4) for b in range(B)])
        ct = np.stack([res[b][f"ct{l}"].reshape(hw, 1) for b in range(B)])
        idx = np.arange(hw, dtype=np.int32)
        coord = np.ascontiguousarray(
            np.broadcast_to(
                np.stack([idx // np.int32(w), idx % np.int32(w)], axis=-1),
                (B, hw, 2),
            )
        )
        out.append((mv, mid, coord, score, bb, ct))
    return tuple(out)


# revision 9
# speedup vs baseline: 1672.0394x; 1672.0394x over previous
"""Trainium2 Bass kernel for FCOSMultiStrideFilter (nms_detection).

Per-level / per-image filter: channel-max + threshold mask + argmax +
masked score/bbox/centerness gather, with the [C, H*W] -> [H*W, C]
layout transpose done on the PE (transpose mode, exact fp32 bit
movement).

Sharding: pure data parallel over the batch dim — core b processes
image b for all 5 FPN levels.

Device layout per core/level (hw positions, CH = 80 cls + 4 bbox + 1 ctr
channels stacked on SBUF partitions):
  - host concatenates cls/bbox/ctr into one [85, hw] tensor per level
  - positions processed in "strided chunks" of 128*K: tile t of chunk c
    holds positions {base + p*K + t | p in 0..127} so that partition p
    owns a contiguous run of K positions -> all DMA-out writes are
    per-partition contiguous.
  - PE transposes [85, 128] -> PSUM [128, 85] (groups of 5 tiles per
    PSUM bank), ACT evacuates PSUM->SBUF, DVE does the channel-max
    reduce + argmax (max_index), GPSIMD applies the threshold mask and
    the small mid/mv fixups.
  - mv/mid/bbox/ctr are accumulated in level-wide SBUF buffers and
    written with one DMA per level; score goes out per chunk (1-2 MB).
  - coord output is input-independent -> computed host-side.
"""

import os
import sys

import numpy as np

_EXTRA_PATHS = [
    "/root/.axon_site",
    "/root/.axon_site/_ro/trn_rl_repo",
    "/root/.axon_site/_ro/pypackages",
    "/opt/trn_rl_repo",
    "/opt/pypackages",
]
for _p in _EXTRA_PATHS:
    if os.path.isdir(_p) and _p not in sys.path:
        sys.path.append(_p)

import concourse.bass as bass  # noqa: E402
import concourse.mybir as mybir  # noqa: E402
from concourse import masks  # noqa: E402
from concourse.bass_utils import run_bass_kernel_spmd  # noqa: E402
from concourse.tile import TileContext  # noqa: E402
from concourse.vector_clock import ScopedClock  # noqa: E402

F32 = mybir.dt.float32
I32 = mybir.dt.int32
U32 = mybir.dt.uint32

B = 8
C = 80
CH = 85  # 80 cls + 4 bbox + 1 ctr rows
LEVELS = [(160, 160), (80, 80), (40, 40), (20, 20), (10, 10)]
THRESHOLD = 0.5
KMAX = 50  # tiles per strided chunk (50*128 = 6400 positions)
GROUP = 5  # transpose tiles per PSUM bank (5*85*4B = 1700B <= 2KB)
TMAX = 200  # max tiles per level (level 0)


# ---------------------------------------------------------------------------
# Walrus in this toolchain accepts only ONE sync-wait command per
# instruction; Tile emits several (final drain + ordinary cross-engine
# deps). Two patches: split the exit-drain waits, and post-process every
# instruction, moving extra waits onto same-engine nops.
# ---------------------------------------------------------------------------
_MAX_WAITS_PER_INST = 1


def _drain_and_barrier_split(self, tick_clock, wait_clock):
    nc = self.nc
    nops = [nc.sync.nop(nofuse=True, hint=f"drain_wait_split_{i}") for i in range(32)]
    drain_inst = nc.sync.drain()
    wait_clock.add_sem_waits(
        drain_inst.ins, ScopedClock({None: tick_clock.global_clock})
    )
    si = drain_inst.ins.sync_info
    if si is not None and si.on_wait and len(si.on_wait) > _MAX_WAITS_PER_INST:
        waits = list(si.on_wait)
        si.on_wait = waits[:_MAX_WAITS_PER_INST]
        extra = waits[_MAX_WAITS_PER_INST:]
        k = 0
        while extra:
            chunk, extra = (
                extra[:_MAX_WAITS_PER_INST],
                extra[_MAX_WAITS_PER_INST:],
            )
            nsi = nops[k].ins.sync_info
            if nsi is None:
                nops[k].ins.sync_info = mybir.SyncInfo(on_wait=chunk, on_update=[])
            else:
                nsi.on_wait = (nsi.on_wait or []) + chunk
            k += 1
            if k >= len(nops):
                raise RuntimeError("too many outstanding sem waits to split")

    nc.all_engine_barrier()
    assert self.sems is not None
    popped = nc._tile_sem_poison_stack.pop()
    assert popped is self._sem_poison
    nc.clear_and_free_semaphores(list(self.sems.allocated().values()))
    nc.all_engine_barrier()


def _install_tile_patch():
    from concourse import tile as _tile

    _tile.TileContext._drain_and_barrier = _drain_and_barrier_split


def _split_multi_waits(nc):
    for f in nc.m.functions:
        for bb in f.blocks:
            insts = bb.instructions
            i = 0
            while i < len(insts):
                inst = insts[i]
                si = inst.sync_info
                if si is not None and si.on_wait and len(si.on_wait) > 1:
                    waits = list(si.on_wait)
                    si.on_wait = waits[:1]
                    extra = waits[1:]
                    for j, wt in enumerate(extra):
                        bi = nc.engines[inst.engine].nop(
                            nofuse=True, hint=f"wait_split_{inst.name}_{j}"
                        )
                        nop_inst = bi.ins
                        nop_inst.sync_info = mybir.SyncInfo(
                            on_wait=[wt], on_update=[]
                        )
                        cur = nc.cur_bb.bb if hasattr(nc.cur_bb, "bb") else None
                        src_list = cur.instructions if cur is not None else None
                        if src_list is not None and src_list and src_list[-1] is nop_inst:
                            src_list.pop()
                        elif insts and insts[-1] is nop_inst:
                            insts.pop()
                        insts.insert(i, nop_inst)
                        i += 1
                i += 1


def _plan_chunks(hw):
    """Strided chunks ('s', pos0, K) covering 128*K positions each (all
    with identical K within a level), plus at most one tail ('t', pos0, w).
    """
    plans = []
    pos = 0
    while hw - pos >= 128 * KMAX:
        plans.append(("s", pos, KMAX))
        pos += 128 * KMAX
    rem = hw - pos
    if rem >= 128:
        k = rem // 128
        plans.append(("s", pos, k))
        pos += 128 * k
        rem = hw - pos
    if rem:
        plans.append(("t", pos, rem))
    return plans


def _groups(k):
    return [(g0, min(GROUP, k - g0)) for g0 in range(0, k, GROUP)]


def build_bass():
    _install_tile_patch()
    nc = bass.Bass()

    xs, mvs, mids, scores, bbs, cts = [], [], [], [], [], []
    for l, (h, w) in enumerate(LEVELS):
        hw = h * w
        xs.append(nc.declare_dram_parameter(f"x{l}", [CH, hw], F32, isOutput=False))
    for l, (h, w) in enumerate(LEVELS):
        hw = h * w
        mvs.append(nc.declare_dram_parameter(f"mv{l}", [hw], F32, isOutput=True))
        mids.append(nc.declare_dram_parameter(f"mid{l}", [hw], I32, isOutput=True))
        scores.append(
            nc.declare_dram_parameter(f"score{l}", [hw, C], F32, isOutput=True)
        )
        bbs.append(nc.declare_dram_parameter(f"bb{l}", [hw, 4], F32, isOutput=True))
        cts.append(nc.declare_dram_parameter(f"ct{l}", [hw, 1], F32, isOutput=True))

    with TileContext(nc) as tc:
        with (
            tc.tile_pool(name="const", bufs=1) as constp,
            tc.tile_pool(name="xin", bufs=2) as xinp,
            tc.tile_pool(name="stage", bufs=2) as stagep,
            tc.tile_pool(name="lvl", bufs=2) as lvlp,
            tc.tile_pool(name="psum", bufs=6, space="PSUM") as psump,
        ):
            ident = constp.tile([128, 128], F32)
            masks.make_identity(nc, ident[:])

            for l in range(5):
                _emit_level(nc, xinp, stagep, lvlp, psump, ident,
                            xs[l], mvs[l], mids[l], scores[l], bbs[l], cts[l],
                            LEVELS[l][0] * LEVELS[l][1])
    _split_multi_waits(nc)
    return nc


def _common_tilegroup(nc, stage, mvb, mkb, idx8, off, pp, k, sc, st):
    """Per PSUM-group compute on `stage[:pp, sc:sc+k, :]` whose tiles map to
    level-buffer columns [off, off+k). `sc`/`st` give the stage-local column
    range. Returns nothing; stage is masked in place."""
    nc.vector.tensor_reduce(
        out=mvb[:pp, off : off + k, 0],
        in_=stage[:pp, sc : sc + k, 0:C],
        axis=mybir.AxisListType.X,
        op=mybir.AluOpType.max,
    )
    nc.gpsimd.tensor_scalar(
        out=mkb[:pp, off : off + k, 0],
        in0=mvb[:pp, off : off + k, 0],
        scalar1=THRESHOLD,
        scalar2=None,
        op0=mybir.AluOpType.is_ge,
    )
    for j in range(k):
        nc.vector.max_index(
            out=idx8[:pp, off + j, :],
            in_max=mvb[:pp, off + j, :].to_broadcast([pp, 8]),
            in_values=stage[:pp, sc + j, 0:C],
        )
    nc.gpsimd.tensor_tensor(
        out=stage[:pp, sc : sc + k, :],
        in0=stage[:pp, sc : sc + k, :],
        in1=mkb[:pp, off : off + k, :].to_broadcast([pp, k, CH]),
        op=mybir.AluOpType.mult,
    )


def _level_fixups(nc, mvb, mkb, idx8, idxf, midi, mvm, pp, t):
    """Batched mv/mid fixups over all t level tiles."""
    nc.gpsimd.tensor_tensor(
        out=mvm[:pp, 0:t, 0],
        in0=mvb[:pp, 0:t, 0],
        in1=mkb[:pp, 0:t, 0],
        op=mybir.AluOpType.mult,
    )
    nc.gpsimd.tensor_copy(out=idxf[:pp, 0:t, 0], in_=idx8[:pp, 0:t, 0])
    nc.gpsimd.tensor_tensor(
        out=idxf[:pp, 0:t, 0],
        in0=idxf[:pp, 0:t, 0],
        in1=mkb[:pp, 0:t, 0],
        op=mybir.AluOpType.mult,
    )
    nc.gpsimd.tensor_tensor(
        out=idxf[:pp, 0:t, 0],
        in0=idxf[:pp, 0:t, 0],
        in1=mkb[:pp, 0:t, 0],
        op=mybir.AluOpType.add,
    )
    # mid = idx*mask + mask - 1  (== idx where mask else -1)
    nc.gpsimd.tensor_scalar(
        out=midi[:pp, 0:t, 0],
        in0=idxf[:pp, 0:t, 0],
        scalar1=-1.0,
        scalar2=None,
        op0=mybir.AluOpType.add,
    )


def _emit_level(nc, xinp, stagep, lvlp, psump, ident, x, mv, mid, score, bb, ct, hw):
    plans = _plan_chunks(hw)
    schunks = [p for p in plans if p[0] == "s"]
    tails = [p for p in plans if p[0] == "t"]
    t_total = sum(k for _, _, k in schunks) + len(tails)

    # level-wide accumulators (column i <-> level tile i)
    mvb = lvlp.tile([128, TMAX, 1], F32, tag="mvb")
    mkb = lvlp.tile([128, TMAX, 1], F32, tag="mkb")
    idx8 = lvlp.tile([128, TMAX, 8], U32, tag="idx8")
    idxf = lvlp.tile([128, TMAX, 1], F32, tag="idxf")
    midi = lvlp.tile([128, TMAX, 1], I32, tag="midi")
    mvm = lvlp.tile([128, TMAX, 1], F32, tag="mvm")
    bbb = lvlp.tile([128, TMAX, 4], F32, tag="bbb")
    ctb = lvlp.tile([128, TMAX, 1], F32, tag="ctb")

    off = 0
    for _, pos0, k in schunks:
        cols = 128 * k
        xt = xinp.tile([CH, 128 * KMAX], F32, tag="xt")
        nc.sync.dma_start(out=xt[:, 0:cols], in_=x[:, pos0 : pos0 + cols])
        xtv = xt[:, 0:cols].rearrange("c (p k) -> c k p", k=k)

        stage = stagep.tile([128, KMAX, CH], F32, tag="stage")
        for g0, ng in _groups(k):
            ps = psump.tile([128, GROUP, CH], F32, tag="ps")
            for j in range(ng):
                nc.tensor.transpose(ps[:, j], xtv[:, g0 + j, :], ident[0:CH, 0:CH])
            nc.scalar.copy(out=stage[:, g0 : g0 + ng, :], in_=ps[:, 0:ng, :])
            _common_tilegroup(
                nc, stage, mvb, mkb, idx8, off + g0, 128, ng, g0, None
            )
        # masked bbox/ctr columns -> level buffers (ACT), score -> DRAM
        nc.scalar.copy(out=bbb[:, off : off + k, :], in_=stage[:, 0:k, C : C + 4])
        nc.scalar.copy(out=ctb[:, off : off + k, :], in_=stage[:, 0:k, C + 4 : CH])
        score_ap = score[pos0 : pos0 + cols, :].rearrange("(p k) c -> p k c", p=128)
        nc.sync.dma_start(out=score_ap, in_=stage[:, 0:k, 0:C])
        off += k

    for _, pos0, w in tails:
        xt = xinp.tile([CH, 128 * KMAX], F32, tag="xt")
        nc.sync.dma_start(out=xt[:, 0:w], in_=x[:, pos0 : pos0 + w])
        stage = stagep.tile([128, KMAX, CH], F32, tag="stage")
        ps = psump.tile([128, GROUP, CH], F32, tag="ps")
        nc.tensor.transpose(ps[0:w, 0], xt[:, 0:w], ident[0:CH, 0:CH])
        nc.scalar.copy(out=stage[0:w, 0, :], in_=ps[0:w, 0, :])
        _common_tilegroup(nc, stage, mvb, mkb, idx8, off, w, 1, 0, None)
        nc.scalar.copy(out=bbb[0:w, off : off + 1, :], in_=stage[0:w, 0:1, C : C + 4])
        nc.scalar.copy(out=ctb[0:w, off : off + 1, :], in_=stage[0:w, 0:1, C + 4 : CH])
        nc.scalar.dma_start(out=score[pos0 : pos0 + w, :], in_=stage[0:w, 0, 0:C])
        off += 1

    _level_fixups(nc, mvb, mkb, idx8, idxf, midi, mvm, 128, t_total)

    # level-wide outputs. Strided chunks share one K, so the whole strided
    # span maps uniformly: position = c*128*K + p*K + t.
    if schunks:
        k = schunks[0][2]
        nch = len(schunks)
        span = nch * 128 * k
        assert all(kk == k for _, _, kk in schunks)
        mv_ap = mv[0:span].rearrange("(c p k) -> p c k", p=128, k=k)
        nc.scalar.dma_start(
            out=mv_ap, in_=mvm[:, 0 : nch * k, 0].rearrange("p (c k) -> p c k", k=k)
        )
        mid_ap = mid[0:span].rearrange("(c p k) -> p c k", p=128, k=k)
        nc.scalar.dma_start(
            out=mid_ap, in_=midi[:, 0 : nch * k, 0].rearrange("p (c k) -> p c k", k=k)
        )
        bb_ap = bb[0:span, :].rearrange("(c p k) x -> p c (k x)", p=128, k=k)
        nc.scalar.dma_start(
            out=bb_ap,
            in_=bbb[:, 0 : nch * k, :].rearrange("p (c k) x -> p c (k x)", k=k),
        )
        ct_ap = ct[0:span, :].rearrange("(c p k) x -> p c (k x)", p=128, k=k)
        nc.scalar.dma_start(
            out=ct_ap,
            in_=ctb[:, 0 : nch * k, :].rearrange("p (c k) x -> p c (k x)", k=k),
        )
    for i, (_, pos0, w) in enumerate(tails):
        toff = (len(schunks) * (schunks[0][2] if schunks else 0)) + i
        mv_ap = mv[pos0 : pos0 + w].rearrange("(p k) -> p k", k=1)
        nc.scalar.dma_start(out=mv_ap, in_=mvm[0:w, toff : toff + 1, 0])
        mid_ap = mid[pos0 : pos0 + w].rearrange("(p k) -> p k", k=1)
        nc.scalar.dma_start(out=mid_ap, in_=midi[0:w, toff : toff + 1, 0])
        nc.scalar.dma_start(out=bb[pos0 : pos0 + w, :], in_=bbb[0:w, toff, :])
        nc.scalar.dma_start(out=ct[pos0 : pos0 + w, :], in_=ctb[0:w, toff, :])


_NC_CACHE = None


def _get_nc():
    global _NC_CACHE
    if _NC_CACHE is None:
        _NC_CACHE = build_bass()
    return _NC_CACHE


def kernel(**inputs):
    nc = _get_nc()

    in_maps = []
    for b in range(B):
        m = {}
        for l, (h, w) in enumerate(LEVELS):
            hw = h * w
            m[f"x{l}"] = np.ascontiguousarray(
                np.concatenate(
                    [
                        np.asarray(inputs[f"cls{l}"][b], np.float32).reshape(C, hw),
                        np.asarray(inputs[f"bbox{l}"][b], np.float32).reshape(4, hw),
                        np.asarray(inputs[f"ctr{l}"][b], np.float32).reshape(1, hw),
                    ],
                    axis=0,
                )
            )
        in_maps.append(m)

    res = run_bass_kernel_spmd(nc, in_maps, list(range(B))).results

    out = []
    for l, (h, w) in enumerate(LEVELS):
        hw = h * w
        mv = np.stack([res[b][f"mv{l}"].reshape(hw) for b in range(B)])
        mid = np.stack(
            [res[b][f"mid{l}"].reshape(hw).astype(np.int32) for b in range(B)]
        )
        score = np.stack([res[b][f"score{l}"].reshape(hw, C) for b in range(B)])
        bb = np.stack([res[b][f"bb{l}"].reshape(hw, 4) for b in range(B)])
        ct = np.stack([res[b][f"ct{l}"].reshape(hw, 1) for b in range(B)])
        idx = np.arange(hw, dtype=np.int32)
        coord = np.ascontiguousarray(
            np.broadcast_to(
                np.stack([idx // np.int32(w), idx % np.int32(w)], axis=-1),
                (B, hw, 2),
            )
        )
        out.append((mv, mid, coord, score, bb, ct))
    return tuple(out)
